# revision 44
# baseline (speedup 1.0000x reference)
"""Trainium2 Bass kernel for nn_MultiHeadAttention_1580547970428.

Sharding: 8 cores = (batch b in 0..3) x (query-half j in 0..1).
Each core computes, for its batch b and its 512 query rows:
  - full k/v projections for batch b (feature-major, via host-transposed inputs)
  - 8-head attention: B-pass [kk, q] orientation feeds attn @ V (with a ones
    column appended to vh so the softmax denominator falls out of the same
    matmul, and exp tiles in bf16); A-pass [q, kk] orientation recomputes the
    logits and produces the fp32 attn output as exp(logits) * recip[q]
  - SPA head gating: partial s reduced on-core, 1KB AllReduce across all 8
    cores, gate softmax, gated head-sum via diagonal-matmul accumulation
  - fc + residual (identity-matmul accumulate) + LayerNorm

All matmuls run in float32r (measured ~7e-6 rel err vs fp32 on HW).
"""

import os

import numpy as np

import concourse.bass as bass
import concourse.mybir as mybir
import concourse.tile as tile
from concourse import bacc
from concourse.bass_utils import run_bass_kernel_spmd

P = 128
H = 8          # heads
DK = 64        # head dim
DM = 512       # d_model
L = 1024       # key length
LQ = 512       # query rows per core
B = 4
N_CORES = 8

F32 = mybir.dt.float32
F32R = mybir.dt.float32r
BF16 = mybir.dt.bfloat16
AF = mybir.ActivationFunctionType
ALU = mybir.AluOpType


def build_program(variant=None, skip_gb=False):
    if variant is None:
        variant = os.environ.get("KVARIANT", "full")
    nc = bacc.Bacc("TRN2", target_bir_lowering=False, num_devices=N_CORES)

    # ---- DRAM I/O (per-core tensors; data differs per core) ----
    qT_d = nc.dram_tensor("qT", [DM, LQ], F32R, kind="ExternalInput")
    kT_d = nc.dram_tensor("kT", [DM, L], F32R, kind="ExternalInput")
    vT_d = nc.dram_tensor("vT", [DM, L], F32R, kind="ExternalInput")
    resid_d = nc.dram_tensor("resid", [LQ, DM], F32R, kind="ExternalInput")
    wqs_d = nc.dram_tensor("wqs", [DM, DM], F32R, kind="ExternalInput")
    wks_d = nc.dram_tensor("wks", [DM, DM], F32R, kind="ExternalInput")
    wvs_d = nc.dram_tensor("wvs", [DM, DM], F32R, kind="ExternalInput")
    wsk_d = nc.dram_tensor("wsk", [DK, DM], F32, kind="ExternalInput")
    wfc_d = nc.dram_tensor("wfc", [DK, DM], F32R, kind="ExternalInput")
    gam_d = nc.dram_tensor("gamma", [1, DM], F32R, kind="ExternalInput")
    bet_d = nc.dram_tensor("beta", [1, DM], F32R, kind="ExternalInput")
    ones_d = nc.dram_tensor("ones", [1, P], F32R, kind="ExternalInput")
    ident_d = nc.dram_tensor("ident", [P, P], F32R, kind="ExternalInput")
    onescol_d = nc.dram_tensor("onescol", [P, H * H], BF16, kind="ExternalInput")
    bsel_d = nc.dram_tensor("bsel", [DK, B], F32, kind="ExternalInput")
    eps_d = nc.dram_tensor("epscol", [P, 1], F32, kind="ExternalInput")

    attn_d = nc.dram_tensor("attn_o", [H, LQ, L], F32, kind="ExternalOutput")
    out_d = nc.dram_tensor("out_o", [LQ, DM], F32, kind="ExternalOutput")

    with tile.TileContext(nc) as tc:
        _body(nc, tc, locals(), variant, skip_gb)
    nc.compile()
    return nc


def _body(nc, tc, t, variant, skip_gb):
    qT_d, kT_d, vT_d = t["qT_d"], t["kT_d"], t["vT_d"]
    resid_d = t["resid_d"]
    wqs_d, wks_d, wvs_d = t["wqs_d"], t["wks_d"], t["wvs_d"]
    wsk_d, wfc_d = t["wsk_d"], t["wfc_d"]
    gam_d, bet_d = t["gam_d"], t["bet_d"]
    ones_d, ident_d, onescol_d, bsel_d = (
        t["ones_d"], t["ident_d"], t["onescol_d"], t["bsel_d"])
    attn_d, out_d = t["attn_d"], t["out_d"]

    with tc.tile_pool(name="persist", bufs=1) as persist:
        _body2(nc, tc, t, persist, variant, skip_gb)


def _body2(nc, tc, t, persist, variant, skip_gb):
    qT_d, kT_d, vT_d = t["qT_d"], t["kT_d"], t["vT_d"]
    resid_d = t["resid_d"]
    wqs_d, wks_d, wvs_d = t["wqs_d"], t["wks_d"], t["wvs_d"]
    wsk_d, wfc_d = t["wsk_d"], t["wfc_d"]
    gam_d, bet_d = t["gam_d"], t["bet_d"]
    ones_d, ident_d, onescol_d, bsel_d = (
        t["ones_d"], t["ident_d"], t["onescol_d"], t["bsel_d"])
    eps_d = t["eps_d"]
    attn_d, out_d = t["attn_d"], t["out_d"]

    # ---- persistent SBUF ----
    qhT_sb = persist.tile([P, 4, LQ], F32R)    # [c, q] head-cols on partitions
    khT_sb = persist.tile([P, 4, L], F32R)     # [c, kk]
    vh_sb = persist.tile([P, H, H, DK + 1], BF16)  # [kk-chunk, h, dv + ones]
    oT_sb = persist.tile([DK, H, LQ], F32R)    # normalized o^T per head
    resid_sb = persist.tile([P, 4, DM], F32R)
    wsk_sb = persist.tile([DK, DM], F32)
    wfc_sb = persist.tile([DK, DM], F32R)
    ones_sb = persist.tile([1, P], F32R)
    ident_sb = persist.tile([P, P], F32R)
    bsel_sb = persist.tile([DK, B], F32)
    eps_sb = persist.tile([P, 1], F32)
    gam_bc = persist.tile([P, DM], F32)
    bet_bc = persist.tile([P, DM], F32)

    nc.sync.dma_start(resid_sb[:], resid_d.ap().rearrange("(qc p) d -> p qc d", p=P))
    nc.sync.dma_start(wsk_sb[:], wsk_d[:])
    nc.sync.dma_start(wfc_sb[:], wfc_d[:])
    nc.sync.dma_start(ones_sb[:], ones_d[:])
    nc.sync.dma_start(ident_sb[:], ident_d[:])
    nc.sync.dma_start(bsel_sb[:], bsel_d[:])
    nc.sync.dma_start(eps_sb[:], eps_d[:])
    nc.sync.dma_start(
        vh_sb[:, :, :, DK : DK + 1],
        onescol_d.ap().rearrange("p (a b) -> p a b", a=H)[:, :, :, None])

    gam_row = persist.tile([1, DM], F32R, tag="grow")
    bet_row = persist.tile([1, DM], F32R, tag="brow")
    nc.sync.dma_start(gam_row[:], gam_d[:])
    nc.sync.dma_start(bet_row[:], bet_d[:])

    # ---------------- Phase 1: projections ----------------
    with (
        tc.tile_pool(name="pin", bufs=1) as pin,
        tc.tile_pool(name="pp", bufs=6, space="PSUM") as pp,
    ):
        # per-dc-chunk tiles so matmuls start as soon as their slice lands
        def load_chunks(dram_t, width, nm):
            tiles = []
            for dc in range(4):
                tl = pin.tile([P, width], F32R, tag=f"{nm}{dc}", name=nm)
                nc.sync.dma_start(
                    tl[:], dram_t.ap()[dc * P : (dc + 1) * P, :])
                tiles.append(tl)
            return tiles

        wqs_c = load_chunks(wqs_d, DM, "wqs")
        qT_c = load_chunks(qT_d, LQ, "qT")
        wks_c = load_chunks(wks_d, DM, "wks")
        kT_c = load_chunks(kT_d, L, "kT")
        wvs_c = load_chunks(wvs_d, DM, "wvs")
        vT_c = load_chunks(vT_d, L, "vT")

        # gamma/beta broadcast across partitions (K=1 outer-product matmul)
        for row, bc in ((gam_row, gam_bc), (bet_row, bet_bc)):
            ps = pp.tile([P, DM], F32, tag="pp", name="pp")
            nc.tensor.matmul(ps[:], ones_sb[0:1, :], row[:], start=True, stop=True)
            nc.any.tensor_copy(bc[:], ps[:])

        # qhT[c, q] = wqs[:, c].T @ qT
        for cc in range(4):
            ps_full = pp.tile([P, DM], F32, tag="pp", name="pp")
            ps = ps_full[:, :LQ]
            for dc in range(4):
                nc.tensor.matmul(
                    ps[:], wqs_c[dc][:, cc * P : (cc + 1) * P], qT_c[dc][:],
                    start=(dc == 0), stop=(dc == 3))
            nc.any.tensor_copy(qhT_sb[:, cc, :], ps[:])

        # khT[c, kk] = wks[:, c].T @ kT
        for cc in range(4):
            for half in range(2):
                ps_full = pp.tile([P, DM], F32, tag="pp", name="pp")
                ps = ps_full[:, :LQ]
                for dc in range(4):
                    nc.tensor.matmul(
                        ps[:], wks_c[dc][:, cc * P : (cc + 1) * P],
                        kT_c[dc][:, half * LQ : (half + 1) * LQ],
                        start=(dc == 0), stop=(dc == 3))
                nc.any.tensor_copy(khT_sb[:, cc, half * LQ : (half + 1) * LQ], ps[:])

        # vh[kk, c] = vT[:, kk].T @ wvs  (natural layout, + ones col separate)
        for kkc in range(H):
            ps = pp.tile([P, DM], F32, tag="pp", name="pp")
            for dc in range(4):
                nc.tensor.matmul(
                    ps[:], vT_c[dc][:, kkc * P : (kkc + 1) * P], wvs_c[dc][:],
                    start=(dc == 0), stop=(dc == 3))
            nc.any.tensor_copy(
                vh_sb[:, kkc, :, 0:DK],
                ps[:].rearrange("p (h d) -> p h d", h=H))

    if variant == "proj":
        return
    n_heads = 1 if variant in ("b0", "a0") else H
    recip_all = persist.tile([1, H, LQ], F32R)
    s_run = persist.tile([DK, 1], F32)

    # ---------------- Phase 2: attention, B/A staggered ----------------
    with (
        tc.tile_pool(name="sb2", bufs=3) as sb2,
        tc.tile_pool(name="attnst", bufs=2) as attnst,
        tc.tile_pool(name="sbc", bufs=1) as sbc,
        tc.tile_pool(name="dram", bufs=1, space="DRAM") as dram,
    ):
        sall_sb = _attn(nc, tc, variant, n_heads, dict(
            qhT_sb=qhT_sb, khT_sb=khT_sb, vh_sb=vh_sb, oT_sb=oT_sb,
            ones_sb=ones_sb, bsel_sb=bsel_sb, recip_all=recip_all,
            s_run=s_run, attn_d=attn_d, sb2=sb2, attnst=attnst, sbc=sbc,
            dram=dram))
        if variant == "cc":
            nc.sync.dma_start(out_d[0:DK, 0:B], sall_sb[:])
            return
        if variant in ("b0", "a0", "attn"):
            return
        _env = dict(
            oT_sb=oT_sb, wsk_sb=wsk_sb, wfc_sb=wfc_sb, ident_sb=ident_sb,
            bsel_sb=bsel_sb, eps_sb=eps_sb, gam_bc=gam_bc, bet_bc=bet_bc,
            resid_sb=resid_sb)
        # ---------------- Phase 3: SPA gate + fc + LN ----------------
        _tail(nc, tc, t, persist, variant, sall_sb, _env, skip_gb)


def _attn(nc, tc, variant, n_heads, env):
    qhT_sb, khT_sb, vh_sb, oT_sb = (
        env["qhT_sb"], env["khT_sb"], env["vh_sb"], env["oT_sb"])
    ones_sb, bsel_sb, recip_all = (
        env["ones_sb"], env["bsel_sb"], env["recip_all"])
    s_run = env["s_run"]
    attn_d = env["attn_d"]
    sb2, attnst, sbc, dram = env["sb2"], env["attnst"], env["sbc"], env["dram"]

    with (
        tc.tile_pool(name="qkb", bufs=2, space="PSUM") as qkb,
        tc.tile_pool(name="av", bufs=2, space="PSUM") as av,
        tc.tile_pool(name="qka", bufs=2, space="PSUM") as qka,
        tc.tile_pool(name="ptiny", bufs=1, space="PSUM") as ptiny,
    ):
        def b_pass(h):
            hp, hc = h % 2, h // 2
            hsl = slice(hp * DK, (hp + 1) * DK)
            oT_ps = av.tile([DK + 1, LQ], F32, tag="oT", name="oT")
            # B-pass: logitsT [kk, q]; exp; attn@V with ones column
            for kkc in range(H):
                qb = qkb.tile([P, LQ], F32, tag="qb", name="qb")
                nc.tensor.matmul(
                    qb[:], khT_sb[hsl, hc, kkc * P : (kkc + 1) * P],
                    qhT_sb[hsl, hc, :], start=True, stop=True)
                e = sb2.tile([P, LQ], BF16, tag="expB", bufs=8, name="e")
                nc.scalar.activation(e[:], qb[:], AF.Exp)
                nc.tensor.matmul(
                    oT_ps[:], vh_sb[:, kkc, h, :], e[:],
                    start=(kkc == 0), stop=(kkc == 7))
            # softmax denominators + normalized o^T
            with nc.allow_low_precision(reason="f32r is ~fp32"):
                nc.vector.reciprocal(
                    recip_all[0:1, h, :], oT_ps[DK : DK + 1, :])
            bc_ps = ptiny.tile([DK, LQ], F32, tag="bc", name="bc")
            nc.tensor.matmul(bc_ps[:], ones_sb[0:1, 0:DK],
                             recip_all[0:1, h, :], start=True, stop=True)
            bc_sb = sb2.tile([DK, LQ], F32, tag="bcs", name="bcs")
            nc.vector.tensor_copy(bc_sb[:], bc_ps[:])
            nc.vector.tensor_tensor(
                oT_sb[:, h, :], oT_ps[0:DK, :], bc_sb[:], ALU.mult)
            # incremental s: fold this head's column-sum into the running sum
            if h == 0:
                nc.vector.reduce_sum(
                    s_run[:], oT_sb[:, 0, :], axis=mybir.AxisListType.X)
            else:
                sh = sb2.tile([DK, 1], F32, tag="sh", name="sh")
                nc.vector.reduce_sum(
                    sh[:], oT_sb[:, h, :], axis=mybir.AxisListType.X)
                nc.vector.tensor_tensor(s_run[:], s_run[:], sh[:], ALU.add)

        def a_pass(h):
            hp, hc = h % 2, h // 2
            hsl = slice(hp * DK, (hp + 1) * DK)
            at = attnst.tile([P, 4 * L], F32, tag="at", name="at")
            at4 = at[:].rearrange("p (qc kk) -> p qc kk", qc=4)
            for qc in range(4):
                rt_ps = ptiny.tile([P, 1], F32, tag="rt", name="rt")
                nc.tensor.matmul(
                    rt_ps[:],
                    recip_all[0:1, h, qc * P : (qc + 1) * P].bitcast(F32),
                    ones_sb[0:1, 0:1].bitcast(F32),
                    start=True, stop=True)
                rts = sb2.tile([P, 1], F32, tag="rts", name="rts")
                nc.vector.tensor_copy(rts[:], rt_ps[:])
                for half in range(2):
                    qa = qka.tile([P, LQ], F32, tag="qa", name="qa")
                    nc.tensor.matmul(
                        qa[:], qhT_sb[hsl, hc, qc * P : (qc + 1) * P],
                        khT_sb[hsl, hc, half * LQ : (half + 1) * LQ],
                        start=True, stop=True)
                    nc.scalar.activation(
                        at4[:, qc, half * LQ : (half + 1) * LQ], qa[:], AF.Exp)
                nc.vector.tensor_scalar_mul(
                    at4[:, qc, :], at4[:, qc, :], rts[:])
            nc.sync.dma_start(
                attn_d.ap().rearrange("h (qc p) kk -> h p qc kk", p=P)[h],
                at4[:])

        def collective():
            sc_sb = sbc.tile([DK, B], F32, tag="sc", name="sc")
            nc.vector.tensor_scalar_mul(sc_sb[:], bsel_sb[:], s_run[:])
            cc_in = dram.tile([DK, B], F32, name="ccin")
            cc_out = dram.tile([DK, B], F32, name="ccout")
            # keep collective traffic off the SP HWDGE queue: a wait there
            # head-of-line-blocks every later attn DMA
            nc.gpsimd.dma_start(cc_in[:], sc_sb[:])
            nc.gpsimd.collective_compute(
                "AllReduce", ALU.add,
                replica_groups=[list(range(N_CORES))],
                ins=[cc_in[:].opt()], outs=[cc_out[:].opt()])
            sall = sbc.tile([DK, B], F32, tag="sall", name="sall")
            nc.gpsimd.dma_start(sall[:], cc_out[:])
            return sall

        if n_heads == 1:
            b_pass(0)
            sall_sb = collective()
            if variant == "a0":
                a_pass(0)
        else:
            # mostly-sequential with two A-passes woven into the late B-phase
            # so the attn-output DMA starts early; the AllReduce still fires
            # right after the last B-pass and hides under the A-pass drain
            for h in range(6):
                b_pass(h)
            a_pass(0)
            b_pass(6)
            a_pass(1)
            b_pass(7)
            sall_sb = collective()
            for h in range(2, H):
                a_pass(h)

        return sall_sb


def _tail(nc, tc, t, persist, variant, sall_sb, env, skip_gb):
    oT_sb = env["oT_sb"]
    wsk_sb, wfc_sb = env["wsk_sb"], env["wfc_sb"]
    ident_sb, bsel_sb, eps_sb = env["ident_sb"], env["bsel_sb"], env["eps_sb"]
    gam_bc, bet_bc = env["gam_bc"], env["bet_bc"]
    resid_sb = env["resid_sb"]
    out_d = t["out_d"]

    with (
        tc.tile_pool(name="ptail", bufs=2, space="PSUM") as ptail,
        tc.tile_pool(name="pfc", bufs=2, space="PSUM") as pfc,
        tc.tile_pool(name="sb3", bufs=2) as sb3,
    ):
        # select own batch column: s = sum(sall * bsel, axis=1)
        smul = sb3.tile([DK, B], F32, tag="smul")
        nc.vector.tensor_tensor(smul[:], sall_sb[:], bsel_sb[:], ALU.mult)
        s_sel = sb3.tile([DK, 1], F32, tag="ssel")
        nc.vector.reduce_sum(s_sel[:], smul[:], axis=mybir.AxisListType.X)

        # gate: z2[dv, h] = wsk[:, h*64+dv].T @ s ; softmax over h
        z2_ps = ptail.tile([DK, H], F32, tag="z2")
        for h in range(H):
            nc.tensor.matmul(
                z2_ps[:, h : h + 1], wsk_sb[:, h * DK : (h + 1) * DK],
                s_sel[:], start=True, stop=True, skip_group_check=True)
        eg_sb = sb3.tile([DK, H], F32, tag="eg")
        acc_sb = sb3.tile([DK, 1], F32, tag="acc")
        nc.scalar.activation(eg_sb[:], z2_ps[:], AF.Exp, accum_out=acc_sb[:])
        rg_sb = sb3.tile([DK, 1], F32, tag="rg")
        nc.vector.reciprocal(rg_sb[:], acc_sb[:])
        g2_sb = sb3.tile([DK, H], F32, tag="g2")
        nc.vector.tensor_scalar_mul(g2_sb[:], eg_sb[:], rg_sb[:])
        if variant == "z2":
            nc.sync.dma_start(out_d[0:DK, 0:H], g2_sb[:])
            return

        # qoT = sum_h diag(g2[:, h]) @ oT_h   (PSUM accumulation)
        qoT_ps = ptail.tile([DK, LQ], F32, tag="qoT")
        for h in range(H):
            dg = sb3.tile([DK, DK], F32R, tag="dg")
            nc.vector.tensor_scalar_mul(
                dg[:], ident_sb[0:DK, 0:DK], g2_sb[:, h : h + 1])
            nc.tensor.matmul(qoT_ps[:], dg[:], oT_sb[:, h, :],
                             start=(h == 0), stop=(h == 7))
        qoT_sb = sb3.tile([DK, LQ], F32R, tag="qoTs")
        nc.vector.tensor_copy(qoT_sb[:], qoT_ps[:])
        if variant == "qo":
            nc.sync.dma_start(out_d[0:DK, :], qoT_sb[:].bitcast(F32))
            return

        # fc + residual + LayerNorm per 128-row chunk
        for qc in range(4):
            fc_ps = pfc.tile([P, DM], F32, tag="fc")
            nc.tensor.matmul(fc_ps[:], qoT_sb[:, qc * P : (qc + 1) * P],
                             wfc_sb[:], start=True, stop=False)
            nc.tensor.matmul(fc_ps[:], ident_sb[:], resid_sb[:, qc, :],
                             start=False, stop=True)
            bn6 = sb3.tile([P, 6], F32, tag="bn6")
            nc.vector.bn_stats(bn6[:], fc_ps[:])
            mv = sb3.tile([P, 2], F32, tag="mv")
            nc.vector.bn_aggr(mv[:], bn6[:])
            sqv = sb3.tile([P, 1], F32, tag="sqv")
            nc.scalar.activation(sqv[:], mv[:, 1:2], AF.Sqrt, bias=eps_sb[:])
            rstd = sb3.tile([P, 1], F32, tag="rstd")
            nc.vector.reciprocal(rstd[:], sqv[:])
            nmr = sb3.tile([P, 1], F32, tag="nmr")
            nc.vector.tensor_scalar(
                nmr[:], mv[:, 0:1], rstd[:], -1.0, ALU.mult, ALU.mult)
            tt = sb3.tile([P, DM], F32, tag="tt")
            nc.scalar.activation(tt[:], fc_ps[:], AF.Identity,
                                 bias=nmr[:], scale=rstd[:])
            if skip_gb:
                nc.sync.dma_start(out_d[qc * P : (qc + 1) * P, :], tt[:])
            else:
                y1 = sb3.tile([P, DM], F32, tag="y1")
                nc.vector.tensor_tensor(y1[:], tt[:], gam_bc[:], ALU.mult)
                y2 = sb3.tile([P, DM], F32, tag="y2")
                nc.vector.tensor_tensor(y2[:], y1[:], bet_bc[:], ALU.add)
                nc.sync.dma_start(out_d[qc * P : (qc + 1) * P, :], y2[:])


_PROGRAM_CACHE = {}


def _get_program(skip_gb=False):
    key = ("nc", skip_gb)
    if key not in _PROGRAM_CACHE:
        _PROGRAM_CACHE[key] = build_program("full", skip_gb=skip_gb)
    return _PROGRAM_CACHE[key]


def kernel(q, k, v, w_qs, w_ks, w_vs, w_sk, w_fc, ln_gamma, ln_beta):
    q = np.asarray(q, dtype=np.float32)
    k = np.asarray(k, dtype=np.float32)
    v = np.asarray(v, dtype=np.float32)
    w_qs = np.asarray(w_qs, dtype=np.float32)
    w_ks = np.asarray(w_ks, dtype=np.float32)
    w_vs = np.asarray(w_vs, dtype=np.float32)
    w_sk = np.asarray(w_sk, dtype=np.float32)
    w_fc = np.asarray(w_fc, dtype=np.float32)
    ln_gamma = np.asarray(ln_gamma, dtype=np.float32)
    ln_beta = np.asarray(ln_beta, dtype=np.float32)

    skip_gb = bool(np.all(ln_gamma == 1.0) and np.all(ln_beta == 0.0))
    nc = _get_program(skip_gb)

    wqs_s = np.ascontiguousarray(w_qs * np.float32(1.0 / 8.0))  # fold 1/sqrt(dk)
    wsk_s = np.ascontiguousarray(w_sk * np.float32(1.0 / 1024.0))  # fold mean over Lq
    gam = np.ascontiguousarray(ln_gamma.reshape(1, DM))
    bet = np.ascontiguousarray(ln_beta.reshape(1, DM))
    ones = np.ones((1, P), dtype=np.float32)
    ident = np.eye(P, dtype=np.float32)
    import ml_dtypes
    onescol = np.ones((P, H * H), dtype=ml_dtypes.bfloat16)
    epscol = np.full((P, 1), 1e-6, dtype=np.float32)

    in_maps = []
    for core in range(N_CORES):
        b, j = divmod(core, 2)
        qs = q[b, j * LQ : (j + 1) * LQ, :]
        bsel = np.zeros((DK, B), dtype=np.float32)
        bsel[:, b] = 1.0
        in_maps.append({
            "qT": np.ascontiguousarray(qs.T),
            "kT": np.ascontiguousarray(k[b].T),
            "vT": np.ascontiguousarray(v[b].T),
            "resid": np.ascontiguousarray(qs),
            "wqs": wqs_s, "wks": w_ks, "wvs": w_vs,
            "wsk": wsk_s, "wfc": w_fc,
            "gamma": gam, "beta": bet,
            "ones": ones, "ident": ident, "onescol": onescol,
            "bsel": bsel,
            "epscol": epscol,
        })

    res = run_bass_kernel_spmd(nc, in_maps, core_ids=list(range(N_CORES)))

    out = np.empty((B, L, DM), dtype=np.float32)
    attn = np.empty((B, H, L, L), dtype=np.float32)
    for core in range(N_CORES):
        b, j = divmod(core, 2)
        out[b, j * LQ : (j + 1) * LQ, :] = res.results[core]["out_o"]
        attn[b, :, j * LQ : (j + 1) * LQ, :] = res.results[core]["attn_o"]
    return out, attn
